# revision 5
# baseline (speedup 1.0000x reference)
"""Multi-head attention (Whisper-style, returns (out, qk)) on 8 TRN2 NeuronCores.

Sharding: batch (4) x head-group (2x8 heads) -> 8 cores.
Per core: x[b] projected with the 512-column slices of Wq/Wk/Wv for its 8
heads; attention is head-local; output projection uses the matching 512
rows of Wo, partial outputs summed on host (+bo).

Self-contained: hardcodes B=4, S=2048, D=1024, H=16.
"""
import numpy as np
from contextlib import ExitStack

import concourse.bass as bass
import concourse.tile as tile
from concourse import bacc, mybir
from concourse.bass_utils import run_bass_kernel_spmd

F32 = mybir.dt.float32
F32R = mybir.dt.float32r
AF = mybir.ActivationFunctionType

B, S, D, H = 4, 2048, 1024, 16
HD = D // H          # 64
HL = 8               # heads per core
DL = HL * HD         # 512 local channels
SCALE = float(HD ** (-0.25))
NEG = -1e9

NSQ = S // 128       # 16 sq tiles of 128
NSB = S // 512       # 4 blocks of 512
NKD = D // 128       # 8 contraction tiles for projections
NCT = DL // 128      # 4 channel tiles


def build():
    nc = bacc.Bacc("TRN2", target_bir_lowering=False, debug=False, num_devices=8)

    xT_d = nc.dram_tensor("xT", [D, S], F32, kind="ExternalInput")
    wq_d = nc.dram_tensor("Wq", [D, DL], F32, kind="ExternalInput")
    wk_d = nc.dram_tensor("Wk", [D, DL], F32, kind="ExternalInput")
    wv_d = nc.dram_tensor("Wv", [D, DL], F32, kind="ExternalInput")
    wo_d = nc.dram_tensor("Wo", [DL, D], F32, kind="ExternalInput")
    bqs_d = nc.dram_tensor("bqs", [128, NCT], F32, kind="ExternalInput")
    bv_d = nc.dram_tensor("bv_row", [1, DL], F32, kind="ExternalInput")
    ones_d = nc.dram_tensor("ones_row", [1, 128], F32, kind="ExternalInput")
    md_d = nc.dram_tensor("mask_diag", [NSQ, 128, 512], F32, kind="ExternalInput")
    mt_d = nc.dram_tensor("maskT_diag", [NSQ, 128, 512], F32, kind="ExternalInput")

    qk_d = nc.dram_tensor("qk_out", [HL, S, S], F32, kind="ExternalOutput")
    out_d = nc.dram_tensor("out_p", [S, D], F32, kind="ExternalOutput")

    with tile.TileContext(nc) as tc, ExitStack() as ctx:
        px = ctx.enter_context(tc.tile_pool(name="px", bufs=8))
        pw = ctx.enter_context(tc.tile_pool(name="pw", bufs=8))
        pqt = ctx.enter_context(tc.tile_pool(name="pqt", bufs=4))
        pkt = ctx.enter_context(tc.tile_pool(name="pkt", bufs=4))
        pv = ctx.enter_context(tc.tile_pool(name="pv", bufs=1))
        pmd = ctx.enter_context(tc.tile_pool(name="pmd", bufs=2))
        pmt = ctx.enter_context(tc.tile_pool(name="pmt", bufs=4))
        pe = ctx.enter_context(tc.tile_pool(name="pe", bufs=2))
        psmall = ctx.enter_context(tc.tile_pool(name="psmall", bufs=2))
        pconst = ctx.enter_context(tc.tile_pool(name="pconst", bufs=1))
        pp = ctx.enter_context(tc.tile_pool(name="pp", bufs=3, space="PSUM"))
        ppav = ctx.enter_context(tc.tile_pool(name="ppav", bufs=2, space="PSUM"))
        ppb = ctx.enter_context(tc.tile_pool(name="ppb", bufs=2, space="PSUM"))

        # ---- constants / small inputs ----
        bqs = pconst.tile([128, NCT], F32, tag="bqs")
        nc.sync.dma_start(bqs[:], bqs_d[:])
        bv_r = pconst.tile([1, DL], F32R, tag="bv")
        nc.gpsimd.dma_start(bv_r[:], bv_d[:])
        ones_r = pconst.tile([1, 128], F32R, tag="onesr")
        nc.gpsimd.dma_start(ones_r[:], ones_d[:])
        ones_f = pconst.tile([1, 64], F32, tag="onesf")
        nc.vector.memset(ones_f[:], 1.0)
        cneg = pconst.tile([128, 512], F32, tag="cneg")
        nc.vector.memset(cneg[:], NEG)

        # ---- phase A: projections ----
        xt = []
        for kd in range(NKD):
            t = px.tile([128, S], F32R, tag="big")
            nc.gpsimd.dma_start(t[:], xT_d[kd * 128:(kd + 1) * 128, :])
            xt.append(t)

        Qt = [pqt.tile([128, S], F32R, tag="qt", name=f"qt{i}") for i in range(NCT)]
        Kt = [pkt.tile([128, S], F32R, tag="kt", name=f"kt{i}") for i in range(NCT)]

        for w_dram, dst, biased in ((wq_d, Qt, True), (wk_d, Kt, False)):
            wt = []
            for kd in range(NKD):
                t = pw.tile([128, DL], F32R, tag="w")
                nc.gpsimd.dma_start(t[:], w_dram[kd * 128:(kd + 1) * 128, :])
                wt.append(t)
            for ct in range(NCT):
                for sb in range(NSB):
                    ps = pp.tile([128, 512], F32, tag="pp")
                    for kd in range(NKD):
                        nc.tensor.matmul(
                            ps[:],
                            wt[kd][:, ct * 128:(ct + 1) * 128],
                            xt[kd][:, sb * 512:(sb + 1) * 512],
                            start=(kd == 0), stop=(kd == NKD - 1),
                        )
                    if biased:
                        nc.scalar.activation(
                            dst[ct][:, sb * 512:(sb + 1) * 512], ps[:],
                            AF.Identity, bias=bqs[:, ct:ct + 1], scale=SCALE)
                    else:
                        nc.scalar.activation(
                            dst[ct][:, sb * 512:(sb + 1) * 512], ps[:],
                            AF.Copy, bias=0.0, scale=SCALE)

        # V in seq-major layout [seq, head, 65] with ones column
        V_sb = pv.tile([128, NSQ, HL, HD + 1], F32R, tag="v")
        wvt = []
        for kd in range(NKD):
            t = pw.tile([128, DL], F32R, tag="w")
            nc.gpsimd.dma_start(t[:], wv_d[kd * 128:(kd + 1) * 128, :])
            wvt.append(t)
        for st in range(NSQ):
            ps = pp.tile([128, 512], F32, tag="pp")
            for kd in range(NKD):
                nc.tensor.matmul(
                    ps[:],
                    xt[kd][:, st * 128:(st + 1) * 128],
                    wvt[kd][:],
                    start=(kd == 0), stop=False,
                )
            nc.tensor.matmul(ps[:], ones_r[:], bv_r[:], start=False, stop=True)
            for h in range(HL):
                nc.vector.tensor_copy(
                    V_sb[:, st, h, 0:HD], ps[:, h * HD:(h + 1) * HD])
        ones_col = pconst.tile([128, NSQ, HL, 1], F32, tag="onescol")
        nc.vector.memset(ones_col[:], 1.0)
        nc.vector.tensor_copy(V_sb[:, :, :, HD:HD + 1], ones_col[:])

        # ---- phase B: qk output pass ([sq, sk] layout) ----
        for i in range(NSQ):
            jmax = i // 4
            md = pmd.tile([128, 512], F32, tag="md")
            nc.sync.dma_start(md[:], md_d[i])
            for h in range(HL):
                ct, po = h // 2, (h % 2) * 64
                strip = px.tile([128, (jmax + 1) * 512], F32, tag="big")
                for j in range(jmax + 1):
                    ps = pp.tile([128, 512], F32, tag="pp")
                    nc.tensor.matmul(
                        ps[:],
                        Qt[ct][po:po + 64, i * 128:(i + 1) * 128],
                        Kt[ct][po:po + 64, j * 512:(j + 1) * 512],
                    )
                    if j == jmax:
                        nc.vector.tensor_add(
                            strip[:, j * 512:(j + 1) * 512], ps[:], md[:])
                    else:
                        nc.vector.tensor_copy(
                            strip[:, j * 512:(j + 1) * 512], ps[:])
                nc.sync.dma_start(
                    qk_d[h, i * 128:(i + 1) * 128, 0:(jmax + 1) * 512], strip[:])
                for j in range(jmax + 1, 4):
                    nc.sync.dma_start(
                        qk_d[h, i * 128:(i + 1) * 128, j * 512:(j + 1) * 512],
                        cneg[:])

        # ---- phase C: E^T = exp(qk^T), AV, normalize ----
        wvT = [px.tile([128, S], F32R, tag="big", name=f"wvT{i}") for i in range(NCT)]
        for b in range(NSB):
            mt = []
            for kk in range(4):
                t = pmt.tile([128, 512], F32, tag="mt")
                nc.sync.dma_start(t[:], mt_d[4 * b + kk])
                mt.append(t)
            kmax = 4 * b + 3
            for h in range(HL):
                ct, po = h // 2, (h % 2) * 64
                av = ppav.tile([HD + 1, 512], F32, tag="av")
                for k in range(kmax + 1):
                    ps = pp.tile([128, 512], F32, tag="pp")
                    nc.tensor.matmul(
                        ps[:],
                        Kt[ct][po:po + 64, k * 128:(k + 1) * 128],
                        Qt[ct][po:po + 64, b * 512:(b + 1) * 512],
                    )
                    if k >= 4 * b:
                        nc.vector.tensor_add(ps[:], ps[:], mt[k - 4 * b][:])
                    E = pe.tile([128, 512], F32R, tag="e")
                    nc.scalar.activation(E[:], ps[:], AF.Exp)
                    nc.tensor.matmul(
                        av[:], V_sb[:, k, h, :], E[:],
                        start=(k == 0), stop=(k == kmax))
                rc = psmall.tile([1, 512], F32, tag="rc")
                nc.vector.reciprocal(rc[:], av[HD:HD + 1, :])
                bc = ppb.tile([64, 512], F32, tag="bc")
                nc.tensor.matmul(bc[:], ones_f[:], rc[:])
                bc_sb = psmall.tile([64, 512], F32, tag="bcs")
                nc.vector.tensor_copy(bc_sb[:], bc[:])
                nc.vector.tensor_mul(
                    wvT[ct][po:po + 64, b * 512:(b + 1) * 512],
                    av[0:HD, :], bc_sb[:])

        # ---- phase D: output projection (partial; host adds pair + bo) ----
        wo = {}
        for ct in range(NCT):
            for jb in range(2):
                t = pw.tile([128, 512], F32R, tag="w")
                nc.gpsimd.dma_start(
                    t[:], wo_d[ct * 128:(ct + 1) * 128, jb * 512:(jb + 1) * 512])
                wo[(ct, jb)] = t
        for st in range(NSQ):
            out_sb = px.tile([128, D], F32, tag="big")
            for jb in range(2):
                ps = pp.tile([128, 512], F32, tag="pp")
                for ct in range(NCT):
                    nc.tensor.matmul(
                        ps[:],
                        wvT[ct][:, st * 128:(st + 1) * 128],
                        wo[(ct, jb)][:],
                        start=(ct == 0), stop=(ct == NCT - 1),
                    )
                nc.vector.tensor_copy(out_sb[:, jb * 512:(jb + 1) * 512], ps[:])
            nc.sync.dma_start(out_d[st * 128:(st + 1) * 128, :], out_sb[:])

    nc.compile()
    return nc


def make_in_maps(x, mask, Wq, bq, Wk, Wv, bv, Wo, bo):
    x = np.ascontiguousarray(x, dtype=np.float32)
    mask = np.ascontiguousarray(mask, dtype=np.float32)
    ones_row = np.ones((1, 128), dtype=np.float32)
    md = np.stack([mask[i * 128:(i + 1) * 128,
                        (i // 4) * 512:((i // 4) + 1) * 512] for i in range(NSQ)])
    maskT = np.ascontiguousarray(mask.T)
    mt = np.stack([maskT[k * 128:(k + 1) * 128,
                         (k // 4) * 512:((k // 4) + 1) * 512] for k in range(NSQ)])
    in_maps = []
    for c in range(8):
        b, hg = c // 2, c % 2
        cols = slice(hg * DL, (hg + 1) * DL)
        bq_l = (bq[cols] * SCALE).astype(np.float32)
        in_maps.append({
            "xT": np.ascontiguousarray(x[b].T),
            "Wq": np.ascontiguousarray(Wq[:, cols], dtype=np.float32),
            "Wk": np.ascontiguousarray(Wk[:, cols], dtype=np.float32),
            "Wv": np.ascontiguousarray(Wv[:, cols], dtype=np.float32),
            "Wo": np.ascontiguousarray(Wo[cols, :], dtype=np.float32),
            "bqs": np.ascontiguousarray(bq_l.reshape(NCT, 128).T),
            "bv_row": np.ascontiguousarray(bv[cols], dtype=np.float32).reshape(1, DL),
            "ones_row": ones_row,
            "mask_diag": md,
            "maskT_diag": mt,
        })
    return in_maps


_NC = None


def get_nc():
    global _NC
    if _NC is None:
        _NC = build()
    return _NC


def assemble(results, bo):
    qk = np.empty((B, H, S, S), dtype=np.float32)
    out = np.empty((B, S, D), dtype=np.float32)
    for c in range(8):
        b, hg = c // 2, c % 2
        qk[b, hg * HL:(hg + 1) * HL] = results[c]["qk_out"]
    for b in range(B):
        out[b] = results[2 * b]["out_p"] + results[2 * b + 1]["out_p"] \
            + bo.astype(np.float32)
    return out, qk


def kernel(x, mask, Wq, bq, Wk, Wv, bv, Wo, bo):
    nc = get_nc()
    in_maps = make_in_maps(x, mask, Wq, bq, Wk, Wv, bv, Wo, bo)
    res = run_bass_kernel_spmd(nc, in_maps, list(range(8)))
    return assemble(res.results, np.asarray(bo))
